# revision 27
# baseline (speedup 1.0000x reference)
"""Causal self-attention (B=4, T=2048, C=1024, H=16) on 8 trn2 cores.

Sharding: batch (4-way) x head-group (2-way).  Core i handles batch i//2 and
heads [8*(i%2), 8*(i%2)+8).  Each core computes qkv projection for its head
slice, causal attention, and a partial out-projection (contraction over its
512 att columns).  Host sums the two partials per batch.

v3 (interleaved emission): the PE HAM clock-gate throttles to 1.2 GHz
whenever the tensor engine micro-idles, and engine queues execute in the
statically scheduled order, so independent work is interleaved at emission
granularity with generators:
  phase A: half-0 projections (serial, DMA-bound lead-in)
  phase B: chunk-0/1 attention pairs round-robined with half-1 projection
           sub-waves (attention exp/DVE latency hides under projection
           matmuls and vice versa)
  phase C: chunk-2 pairs round-robined with chunk-0/1 out-projections
  phase D: chunk-3 pairs round-robined with chunk-2 out-projections
  phase E: chunk-3 out-projection
PSUM: one shared pool of 2-bank tiles (bufs=3: score tiles / out-proj
accumulators / sums-broadcast / projection sub-waves, each holding one) + 2
single-bank AV accumulators.  ACT does exp + sums-row copies only; all other
psum evacuation is DVE.  Everything is bf16 except the f32 psum paths and
the normalization chain (projection inputs bf16 halves the startup DMA).

Layouts on chip (same as v1):
  - QT/KT  [128, 4, T]   rows = head-major (hl*64+d), T on free dim
  - V      [128, 16, 772]: per t-tile, per head pair [V_e|1] + [1|0*63|V_o]
  - attT   [128, 4, T]   rows = c_local = hl*64+d  (lhsT for out-proj)
Softmax sums come from the appended ones columns in V (even head: psum row
64; odd head: row 0), then ACT copy -> PE ones-broadcast -> DVE reciprocal
-> DVE normalize into attT.
"""

import numpy as np
import ml_dtypes

import concourse.bass as bass
import concourse.mybir as mybir
import concourse.tile as tile
from concourse import bacc, bass_isa, bass_utils

B, T, C, H, HD = 4, 2048, 1024, 16, 64
HG = 2  # head groups (tensor-parallel dim)
HPG = H // HG  # 8 heads per group
OG = HPG * HD  # 512: local width of q/k/v slice
KT_C = C // 128  # 8 contraction tiles for the projections
NT = T // 128  # 16 t-tiles
NQ = T // 512  # 4 tq chunks
PAIR_W = 65 + 128  # v_sb cols per head pair: [V_e|1] + [0*63|1|V_o]

f32 = mybir.dt.float32
f32r = mybir.dt.float32r
bf16 = mybir.dt.bfloat16
BF16 = ml_dtypes.bfloat16

TRACE = False  # test.py flips this for profiling runs
DEBUG = False  # adds intermediate dumps (qt/kt/v/attT) as extra outputs
LAST_RUN = {}

_NC_CACHE = []


def _mm(nc, out, lhsT, rhs, **kw):
    nc.tensor.matmul(out, lhsT, rhs, **kw)


def _build_nc():
    nc = bacc.Bacc(trn_type="TRN2", target_bir_lowering=False, debug=False)
    xT = nc.dram_tensor("xT", [C, T], bf16, kind="ExternalInput").ap()
    wqk = nc.dram_tensor("wqk", [8, 128, 1024], bf16, kind="ExternalInput").ap()
    wv = nc.dram_tensor("wv", [C, OG], bf16, kind="ExternalInput").ap()
    wo = nc.dram_tensor("wo", [OG, C], bf16, kind="ExternalInput").ap()
    masks = nc.dram_tensor("masks", [128, 1280], bf16, kind="ExternalInput").ap()
    y = nc.dram_tensor("y", [T, C], f32, kind="ExternalOutput").ap()
    dbg = None
    if DEBUG:
        dbg = {
            "qt": nc.dram_tensor("dbg_qt", [128, 4, T], bf16, kind="ExternalOutput").ap(),
            "kt": nc.dram_tensor("dbg_kt", [128, 4, T], bf16, kind="ExternalOutput").ap(),
            "v": nc.dram_tensor("dbg_v", [128, NT, 4 * PAIR_W], bf16, kind="ExternalOutput").ap(),
            "attT": nc.dram_tensor("dbg_attT", [128, 4, T], bf16, kind="ExternalOutput").ap(),
        }

    with tile.TileContext(nc) as tc:
        _body(tc, nc, xT, wqk, wv, wo, masks, y, dbg)
    nc.compile()
    return nc


def _drive(*gens):
    """Round-robin the generators until all are exhausted."""
    live = list(gens)
    while live:
        nxt = []
        for g in live:
            try:
                next(g)
                nxt.append(g)
            except StopIteration:
                pass
        live = nxt


def _body(tc, nc, xT, wqk, wv, wo, masks, y, dbg):
    exp_f = mybir.ActivationFunctionType.Exp

    with (
        tc.tile_pool(name="persist", bufs=1) as persist,
        tc.tile_pool(name="wv_p", bufs=1) as wv_p,
        tc.tile_pool(name="xh_p", bufs=1) as xh_p,
        tc.tile_pool(name="wqk_p", bufs=1) as wqk_p,
        tc.tile_pool(name="mask_p", bufs=1) as mask_p,
        tc.tile_pool(name="wo_p", bufs=1) as wo_p,
        tc.tile_pool(name="pt_p", bufs=3) as pt_p,
        tc.tile_pool(name="sums_p", bufs=1) as sums_p,
        tc.tile_pool(name="bcast_p", bufs=1) as bcast_p,
        tc.tile_pool(name="yo_p", bufs=2) as yo_p,
        tc.tile_pool(name="st_ps", bufs=2, space="PSUM") as st_ps,
        tc.tile_pool(name="rot_ps", bufs=2, space="PSUM") as rot_ps,
        tc.tile_pool(name="av_ps", bufs=2, space="PSUM") as av_ps,
    ):
        qt = persist.tile([128, 4, T], bf16)
        kt = persist.tile([128, 4, T], bf16)
        v_sb = persist.tile([128, NT, 4 * PAIR_W], bf16)
        attT = persist.tile([128, 4, T], bf16)

        mk = mask_p.tile([128, 1280], bf16)
        wo_sb = wo_p.tile([128, 4, C], bf16)
        wv_sb = wv_p.tile([128, KT_C, OG], bf16)
        wts = {}
        # softmax-sums staging rows + the all-ones lhsT for the PE
        # partition-broadcast matmuls (built on-chip, no DMA input; f32r
        # memset is not a valid ISA op so memset f32 then cast)
        sums_tiles = [
            sums_p.tile([128, 512], f32r, tag="sm0", name="sm0"),
            sums_p.tile([128, 512], f32r, tag="sm1", name="sm1"),
        ]
        ones_f = sums_p.tile([128, 128], f32, tag="ones_f", name="ones_f")
        nc.vector.memset(ones_f[:], 1.0)
        ones_sb = sums_p.tile([128, 128], f32r, tag="ones", name="ones_sb")
        nc.vector.tensor_copy(ones_sb[:], ones_f[:])

        # ================= projections (one half of T) =================
        # k-outer: each k-step of the contraction only needs xT k-slice k,
        # so matmuls start as soon as the first DMA lands.  Sub-waves hold a
        # single 2-bank psum tile so the shared pool stays fluid for the
        # interleaved attention chunks.
        def emit_half(half):
            t0 = half * (T // 2)
            xs = []
            for k in range(KT_C):
                xt = xh_p.tile(
                    [128, T // 2], bf16, tag=f"xh{k}", name=f"xh{half}_{k}"
                )
                xs.append(xt)

            def load_xh(k):
                nc.sync.dma_start(
                    xs[k][:], xT[k * 128 : (k + 1) * 128, t0 : t0 + T // 2]
                )

            if half == 0:
                # priority order: first k-slice + first-wave weights (split
                # column-wise so the first matmuls aren't gated on one big
                # transfer), then the rest, then V weights / v_sb init.
                for c4 in range(4):
                    nc.sync.dma_start(
                        xs[0][:, c4 * 256 : (c4 + 1) * 256],
                        xT[0:128, t0 + c4 * 256 : t0 + (c4 + 1) * 256],
                    )
                wt = wqk_p.tile([128, 1024], bf16, tag="wqk0", name="wt0")
                for c4 in range(4):
                    nc.sync.dma_start(
                        wt[:, c4 * 256 : (c4 + 1) * 256],
                        wqk[0, :, c4 * 256 : (c4 + 1) * 256],
                    )
                wts[0] = wt
                for m in range(1, 4):
                    wt = wqk_p.tile([128, 1024], bf16, tag=f"wqk{m}", name=f"wt{m}")
                    nc.sync.dma_start(wt[:], wqk[m, :, :])
                    wts[m] = wt
                for k in range(1, KT_C):
                    load_xh(k)
                for m in range(4, 8):
                    wt = wqk_p.tile([128, 1024], bf16, tag=f"wqk{m}", name=f"wt{m}")
                    nc.sync.dma_start(wt[:], wqk[m, :, :])
                    wts[m] = wt
                for k in range(KT_C):
                    nc.sync.dma_start(wv_sb[:, k, :], wv[k * 128 : (k + 1) * 128, :])
                # v_sb ones/zeros bands: cols [64:129) of each pair are the
                # even/odd sums columns (64,65 = 1) + the odd-head zero block
                vz = v_sb[:].rearrange("p t (q w) -> p (t q) w", q=4, w=PAIR_W)
                nc.vector.memset(vz[:, :, 66:129], 0.0)
                nc.vector.memset(vz[:, :, 64:66], 1.0)
                nc.sync.dma_start(mk[:], masks[:])
                for k in range(4):
                    nc.sync.dma_start(wo_sb[:, k, :], wo[k * 128 : (k + 1) * 128, :])
            else:
                for k in range(KT_C):
                    load_xh(k)
            yield

            for wave in range(2):  # A: q (m 0..3), B: k (m 4..7)
                dst = qt if wave == 0 else kt
                for sub in range(4):
                    m = wave * 4 + sub
                    for n in range(2):
                        pw = rot_ps.tile(
                            [128, 512], f32, tag="rot", name=f"pw{half}_{m}_{n}"
                        )
                        for k in range(KT_C):
                            _mm(
                                nc,
                                pw[:],
                                wts[m][:, k * 128 : (k + 1) * 128],
                                xs[k][:, n * 512 : (n + 1) * 512],
                                start=(k == 0),
                                stop=(k == KT_C - 1),
                            )
                        nc.vector.tensor_copy(
                            dst[:, sub, t0 + n * 512 : t0 + (n + 1) * 512],
                            pw[:],
                        )
                    yield

            # V wave: out rows t, free = o (head-major)
            for tl in range(8):
                tt = half * 8 + tl
                pv = rot_ps.tile(
                    [128, 512], f32, tag="rot", name=f"pv{half}_{tl}"
                )
                for k in range(KT_C):
                    _mm(
                        nc,
                        pv[:],
                        xs[k][:, tl * 128 : (tl + 1) * 128],
                        wv_sb[:, k, :],
                        start=(k == 0),
                        stop=(k == KT_C - 1),
                    )
                src = pv[:].rearrange("p (h d) -> p h d", d=64)
                dstv = v_sb[:, tt, :].rearrange("p (q w) -> p q w", w=PAIR_W)
                nc.vector.tensor_copy(dstv[:, :, 0:64], src[:, 0::2, :])
                nc.vector.tensor_copy(dstv[:, :, 129:193], src[:, 1::2, :])
                if tl % 2 == 1:
                    yield

        # ================= attention =================
        def head_ctx(hl):
            """Slices/layout facts for local head hl."""
            p0 = (hl % 2) * 64
            mt = hl // 2
            qrow = slice(p0, p0 + 64)
            vb0 = (hl // 2) * PAIR_W
            if hl % 2 == 0:
                vsl = (vb0, vb0 + 65)  # [V|1] -> rows 0..64
                srow, arow = 64, slice(0, 64)
            else:
                vsl = (vb0 + 65, vb0 + 193)  # [1|0*63|V] -> row 0 sums, 64..127 att
                srow, arow = 0, slice(64, 128)
            return p0, mt, qrow, vsl, srow, arow

        # deferred normalization: each pair's norm chain is emitted at the
        # START of the next pair (after its first QKs), so the in-order PE
        # queue never stalls at a broadcast matmul waiting on the ACT sums
        # copy -- the next pair's QK tiles are already ahead of it.
        pending = {}

        def flush_norm():
            if not pending:
                return
            avs_p, ctxs_p, tq_p, tag = pending.pop("n")
            for s in (0, 1):
                _, mt, _, _, srow, arow = ctxs_p[s]
                sm = sums_tiles[s]
                nc.scalar.copy(
                    sm[srow : srow + 1, :], avs_p[s][srow : srow + 1, :]
                )
                bps = rot_ps.tile(
                    [128, 512], f32, tag="rot", name=f"bps_{tag}_{s}"
                )
                _mm(
                    nc,
                    bps[:],
                    ones_sb[srow : srow + 1, :],
                    sm[srow : srow + 1, :],
                    start=True,
                    stop=True,
                )
                bc = bcast_p.tile([128, 512], f32, tag=f"bc{s}")
                nc.vector.reciprocal_approx_fast(bc[:], bps[:])
                nc.vector.tensor_mul(
                    attT[arow, mt, tq_p], avs_p[s][arow, :], bc[arow, :]
                )

        def do_chunk(j):
            ntk = 4 * j + 4
            ng = ntk // 2
            tq = slice(j * 512, (j + 1) * 512)
            for ha in range(0, HPG, 2):
                ctxs = [head_ctx(ha), head_ctx(ha + 1)]
                pts = {0: [None] * ng, 1: [None] * ng}

                def emit_pair(s, g):
                    _, mt, qrow, _, _, _ = ctxs[s]
                    # diagonal tiles only need tq >= tk: narrow the
                    # st/exp/av width (512/384/256/128) instead of masking
                    # fully-computed tiles.
                    geom = []  # per u: (tq_off, width, pt_col)
                    pcol = 0
                    for u in range(2):
                        tk = 2 * g + u
                        v = tk - 4 * j
                        off = 128 * v if v > 0 else 0
                        w = 512 - off
                        if u == 1 and pcol == 512:
                            pcol = 512  # second slot starts at bank 1
                        geom.append((off, w, pcol))
                        pcol = 512 if u == 0 and w == 512 else pcol + w
                    dg = 2 * g - 4 * j
                    ps = st_ps.tile(
                        [128, 1024], f32, tag="st", name=f"st_{j}_{ha}_{s}_{g}"
                    )
                    for u in range(2):
                        off, w, pc = geom[u]
                        tk = 2 * g + u
                        _mm(
                            nc,
                            ps[:, pc : pc + w],
                            kt[qrow, mt, tk * 128 : (tk + 1) * 128],
                            qt[qrow, mt, j * 512 + off : (j + 1) * 512],
                            start=True,
                            stop=True,
                        )
                    tot = geom[1][2] + geom[1][1]
                    pt = pt_p.tile([128, 1024], bf16, tag=f"pt{s}")
                    nc.scalar.activation(
                        pt[:, 0:tot], ps[:, 0:tot], exp_f, scale=0.125
                    )
                    if dg == 0:  # pair (4j, 4j+1): widths 512|384
                        nc.vector.tensor_mul(
                            pt[:, 0:896], pt[:, 0:896], mk[:, 0:896]
                        )
                    elif dg == 2:  # pair (4j+2, 4j+3): widths 256|128
                        nc.vector.tensor_mul(
                            pt[:, 0:384], pt[:, 0:384], mk[:, 896:1280]
                        )
                    pts[s][g] = (pt, geom)

                emit_pair(0, 0)
                emit_pair(1, 0)
                flush_norm()
                avs = [
                    av_ps.tile([128, 512], f32, tag="av", name=f"av{s}_{ha}_{j}")
                    for s in (0, 1)
                ]
                for g in range(ng):
                    if g + 1 < ng:
                        emit_pair(0, g + 1)
                        emit_pair(1, g + 1)
                    for u in range(2):
                        for s in (0, 1):
                            _, _, _, vsl, _, _ = ctxs[s]
                            pt, geom = pts[s][g]
                            off, w, pc = geom[u]
                            tk = 2 * g + u
                            _mm(
                                nc,
                                avs[s][0 : vsl[1] - vsl[0], off : off + w],
                                v_sb[:, tk, vsl[0] : vsl[1]],
                                pt[:, pc : pc + w],
                                start=(tk == 0),
                                stop=(tk == ntk - 1),
                            )

                pending["n"] = (avs, ctxs, tq, f"{j}_{ha}")
                yield

        def do_outproj_chunk(j):
            # y rows for tq chunk j: 4 t-tiles x 2 o-halves
            for tl in range(4):
                tt = 4 * j + tl
                for o in range(2):
                    yps = rot_ps.tile(
                        [128, 512], f32, tag="rot", name=f"yps_{tt}_{o}"
                    )
                    for k in range(4):
                        _mm(
                            nc,
                            yps[:],
                            attT[:, k, tt * 128 : (tt + 1) * 128],
                            wo_sb[:, k, o * 512 : (o + 1) * 512],
                            start=(k == 0),
                            stop=(k == 3),
                        )
                    yo = yo_p.tile([128, 512], f32, tag="yo", name=f"yo_{tt}_{o}")
                    nc.vector.tensor_copy(yo[:], yps[:])
                    nc.sync.dma_start(
                        y[tt * 128 : (tt + 1) * 128, o * 512 : (o + 1) * 512],
                        yo[:],
                    )
                yield

        def chain(*gens):
            for g in gens:
                yield from g

        # phase A: half-0 projections (serial)
        for _ in emit_half(0):
            pass
        # phase B: chunk-0/1 attention interleaved with half-1 projections
        _drive(chain(do_chunk(0), do_chunk(1)), emit_half(1))
        # phase C: chunk-2 pairs interleaved with chunk-0/1 out-projections
        _drive(do_chunk(2), chain(do_outproj_chunk(0), do_outproj_chunk(1)))
        # phase D: chunk-3 pairs interleaved with chunk-2 out-projection
        _drive(do_chunk(3), do_outproj_chunk(2))
        # phase E: chunk-3 out-projection (last pair's norm first)
        flush_norm()
        for _ in do_outproj_chunk(3):
            pass

        if dbg is not None:
            for mm_ in range(4):
                nc.sync.dma_start(dbg["qt"][:, mm_, :], qt[:, mm_, :])
                nc.sync.dma_start(dbg["kt"][:, mm_, :], kt[:, mm_, :])
                nc.sync.dma_start(dbg["attT"][:, mm_, :], attT[:, mm_, :])
            for tt_ in range(NT):
                nc.sync.dma_start(dbg["v"][:, tt_, :], v_sb[:, tt_, :])


def _round_fp32r(a):
    """Round fp32 to the fp32r grid (11 mantissa bits; low 12 bits zero), RNE."""
    u = np.ascontiguousarray(a, dtype=np.float32).view(np.uint32)
    lsb = (u >> 12) & 1
    out = ((u + 0x7FF + lsb) & 0xFFFFF000).astype(np.uint32)
    return out.view(np.float32)


def _host_prep(x, w_qkv, w_out):
    xT_all = np.ascontiguousarray(x.transpose(0, 2, 1)).astype(BF16)
    # packed diagonal masks, all variant-0 (keep iff tq_local >= tk_local):
    # [0:512) pair1-u0 w=512, [512:896) pair1-u1 w=384,
    # [896:1152) pair2-u0 w=256, [1152:1280) pair2-u1 w=128
    tk_l = np.arange(128)[:, None]
    m0 = (np.arange(512)[None, :] >= tk_l).astype(BF16)
    masks = np.concatenate([m0, m0[:, :384], m0[:, :256], m0[:, :128]], axis=1)

    per_group = []
    for g in range(HG):
        wq = w_qkv[g * OG : (g + 1) * OG]
        wk = w_qkv[C + g * OG : C + (g + 1) * OG]
        wvg = w_qkv[2 * C + g * OG : 2 * C + (g + 1) * OG]
        wqkT = np.concatenate([wq, wk], axis=0).T  # (C, 1024)
        # wqk_r[m, p, k*128+j] = wqkT[k*128+p, m*128+j]
        wqk_r = np.ascontiguousarray(
            wqkT.reshape(8, 128, 8, 128).transpose(2, 1, 0, 3).reshape(8, 128, 1024)
        ).astype(BF16)
        wv_t = np.ascontiguousarray(wvg.T).astype(BF16)  # (C, 512)
        wo_t = np.ascontiguousarray(w_out.T[g * OG : (g + 1) * OG]).astype(
            BF16
        )  # (512, C)
        per_group.append((wqk_r, wv_t, wo_t))
    return xT_all, masks, per_group


def kernel(x, w_qkv, w_out):
    x = np.asarray(x)
    w_qkv = np.asarray(w_qkv)
    w_out = np.asarray(w_out)
    xT_all, masks, per_group = _host_prep(x, w_qkv, w_out)

    if not _NC_CACHE:
        _NC_CACHE.append(_build_nc())
    nc = _NC_CACHE[0]

    in_maps = []
    for core in range(8):
        b, g = core // 2, core % 2
        wqk_r, wv_t, wo_t = per_group[g]
        in_maps.append(
            {"xT": xT_all[b], "wqk": wqk_r, "wv": wv_t, "wo": wo_t,
             "masks": masks}
        )

    res = bass_utils.run_bass_kernel_spmd(
        nc, in_maps, core_ids=list(range(8)), trace=TRACE
    )
    LAST_RUN["res"] = res

    y = np.empty((B, T, C), np.float32)
    for b in range(B):
        y[b] = res.results[2 * b]["y"] + res.results[2 * b + 1]["y"]
    return y
